# revision 5
# baseline (speedup 1.0000x reference)
"""Conditional NT-Xent loss kernel V3 for Trainium2 (8 NeuronCores, SPMD).

Per chunk of 2 consecutive rows from each of zjs/zis (a,b = zjs rows; c,d = zis
rows): need 4 squared norms + 6 pairwise dots, cos_xy = s_xy/(|x||y|),
logits = 2*cos, per-row loss = lse(3 logits) - pos, total = sum / B.

V3 pipeline (vs V2): no PSUM->SBUF materialization; DVE products read the
PE-transposed bf16 tiles directly from PSUM (keeps 2x DVE mode, +65ns/op);
squares split ACT/Pool; reductions stay as cheap PE ones-matmuls writing a
f32-bitcast head of the (already consumed) PSUM tile; one act-table load.

Sharding: batch(chunk)-parallel across 8 cores; each core computes a partial
sum [128,1]; host sums partials and divides by B.
"""

import numpy as np

import concourse.bass as bass
import concourse.tile as tile
from concourse import bacc, masks, mybir
from concourse.bass_utils import run_bass_kernel_spmd

N_CORES = 8
B_FULL = 65536            # total rows in zis (== zjs)
ROWS = B_FULL // N_CORES  # 8192 rows per core shard
D = 256
GROUPS = 32               # groups of 128 chunks per core
GS = 2                    # groups per set: PT [128,2048] bf16 = 2 PSUM banks
SETS = GROUPS // GS
F32 = mybir.dt.float32
BF16 = mybir.dt.bfloat16
ALU = mybir.AluOpType
ACTF = mybir.ActivationFunctionType

# stats tile S[:, g*10 + t]
# t: 0=na 1=nb 2=nc 3=nd 4=ab 5=cd 6=ac 7=bd 8=ad 9=bc


def _epilogue(tc, nc, epi, S, out):
    """Per-chunk softmax math on stats S [128, GROUPS*10] -> out [128,1]."""
    Sv = S[:].rearrange("p (g t) -> p g t", t=10)
    norms = Sv[:, :, 0:4]
    svals = Sv[:, :, 4:10]
    G = GROUPS

    LN = epi.tile([128, G * 4], F32, tag="ln")
    LNv = LN[:].rearrange("p (g t) -> p g t", t=4)
    nc.scalar.activation(LNv, norms, ACTF.Ln)

    # q_xy = ln nx + ln ny (pair order ab cd ac bd ad bc)
    Q = epi.tile([128, G * 6], F32, tag="q")
    Qv = Q[:].rearrange("p (g t) -> p g t", t=6)
    pair_norm_idx = [(0, 1), (2, 3), (0, 2), (1, 3), (0, 3), (1, 2)]
    for t, (x, y) in enumerate(pair_norm_idx):
        nc.vector.tensor_add(
            Qv[:, :, t : t + 1], LNv[:, :, x : x + 1], LNv[:, :, y : y + 1]
        )

    # rr_xy = exp(-0.5 q) = 1/(|x||y|)
    RQ = epi.tile([128, G * 6], F32, tag="rq")
    RQv = RQ[:].rearrange("p (g t) -> p g t", t=6)
    nc.scalar.activation(RQv, Qv, ACTF.Exp, scale=-0.5)

    # cos_xy = s_xy * rr_xy
    C = epi.tile([128, G * 6], F32, tag="cos")
    Cv = C[:].rearrange("p (g t) -> p g t", t=6)
    nc.vector.tensor_mul(Cv, svals, RQv)

    # E_xy = exp(2 cos)
    E = epi.tile([128, G * 6], F32, tag="e")
    Ev = E[:].rearrange("p (g t) -> p g t", t=6)
    nc.scalar.activation(Ev, Cv, ACTF.Exp, scale=2.0)

    def ecol(t):
        return Ev[:, :, t : t + 1]

    # softmax denominators for the 4 rows of each chunk
    DEN = epi.tile([128, G * 4], F32, tag="den")
    DENv = DEN[:].rearrange("p (g t) -> p g t", t=4)
    TMP = epi.tile([128, G * 4], F32, tag="tmp")
    TMPv = TMP[:].rearrange("p (g t) -> p g t", t=4)
    den_terms = [
        (0, 4, 2),  # D0 = (ab + ad) + ac
        (0, 5, 3),  # D1 = (ab + bc) + bd
        (5, 1, 2),  # D2 = (bc + cd) + ac
        (4, 1, 3),  # D3 = (ad + cd) + bd
    ]
    for r, (u, v, w) in enumerate(den_terms):
        nc.vector.tensor_add(TMPv[:, :, r : r + 1], ecol(u), ecol(v))
        nc.vector.tensor_add(DENv[:, :, r : r + 1], TMPv[:, :, r : r + 1], ecol(w))

    LD = epi.tile([128, G * 4], F32, tag="ld")
    LDv = LD[:].rearrange("p (g t) -> p g t", t=4)
    nc.scalar.activation(LDv, DENv, ACTF.Ln)

    LG = epi.tile([128, G], F32, tag="lg")
    nc.vector.reduce_sum(
        LG[:].rearrange("p (g o) -> p g o", o=1), LDv, axis=mybir.AxisListType.X
    )

    T1 = epi.tile([128, G], F32, tag="t1")
    nc.vector.tensor_add(
        T1[:].rearrange("p (g o) -> p g o", o=1), Cv[:, :, 2:3], Cv[:, :, 3:4]
    )

    # loss per chunk-col = LG - 4*T1
    LC = epi.tile([128, G], F32, tag="lc")
    nc.vector.scalar_tensor_tensor(
        out=LC[:], in0=T1[:], scalar=-4.0, in1=LG[:], op0=ALU.mult, op1=ALU.add
    )

    ACC = epi.tile([128, 1], F32, tag="acc")
    nc.vector.reduce_sum(ACC[:], LC[:], axis=mybir.AxisListType.X)
    nc.sync.dma_start(out=out, in_=ACC[:])


def _trace_kernel(tc, nc, zjs, zis, out):
    # chunk-major natural views: partition = chunk-in-group, free = (g, two, f)
    zjs_v = zjs.rearrange("(g p two) f -> p g (two f)", p=128, two=2)
    zis_v = zis.rearrange("(g p two) f -> p g (two f)", p=128, two=2)

    with (
        tc.tile_pool(name="consts", bufs=1) as consts,
        tc.tile_pool(name="nat", bufs=1) as nat,
        tc.tile_pool(name="tp", bufs=3, space="PSUM") as tp,
        tc.tile_pool(name="red", bufs=2, space="PSUM") as red,
        tc.tile_pool(name="prod", bufs=6) as prod,
        tc.tile_pool(name="sq", bufs=4) as sqp,
        tc.tile_pool(name="stats", bufs=1) as stats,
        tc.tile_pool(name="epi", bufs=1) as epi,
    ):
        ident = consts.tile([128, 128], BF16)
        masks.make_identity(nc, ident[:])
        ones = consts.tile([128, 1], BF16)
        nc.gpsimd.memset(ones[:], 1.0)

        # full-shard natural loads, bf16 cast on DMA; staggered spans so the
        # first set can start early while later, larger loads stream in
        ZJ = nat.tile([128, GROUPS * 512], BF16)
        ZI = nat.tile([128, GROUPS * 512], BF16)
        spans = [(0, 2), (2, 8), (8, 16), (16, 24), (24, 32)]
        for g0, g1 in spans:
            for Z, zv in ((ZJ, zjs_v), (ZI, zis_v)):
                nc.gpsimd.dma_start(
                    out=Z[:, g0 * 512 : g1 * 512].rearrange(
                        "p (g tf) -> p g tf", g=g1 - g0
                    ),
                    in_=zv[:, g0:g1, :],
                )

        S = stats.tile([128, GROUPS * 10], F32)
        W = GS * 512  # per-input col width in PT

        def emit_transposes(si):
            PT = tp.tile([128, 2 * W], BF16, tag="pt")
            for i, Z in ((0, ZJ), (1, ZI)):
                for g in range(GS):
                    base = (i * GS + g) * 512
                    nbase = (si * GS + g) * 512
                    for k in range(4):
                        nc.tensor.transpose(
                            PT[:, base + 128 * k : base + 128 * (k + 1)],
                            Z[:, nbase + 128 * k : nbase + 128 * (k + 1)],
                            ident[:],
                        )
            return PT

        def emit_body(si, PT):
            g0 = si * GS

            # materialize PSUM->SBUF: hardware allows at most ONE PSUM input
            # per compute instruction, so products need an SBUF copy. Split
            # the copy across ACT (bulk) and DVE (tail); squares then read
            # the SBUF copy so they can run on DVE/Pool too.
            TC = prod.tile([128, 2 * W], BF16, tag="tc")
            nc.vector.tensor_copy(TC[:, 0:768], PT[:, 0:768])
            nc.scalar.activation(TC[:, 768 : 2 * W], PT[:, 768 : 2 * W], ACTF.Copy)

            TC5 = TC[:].rearrange("p (i g r sc) -> p i g r sc", i=2, g=GS, r=2)
            Jv = TC[:, 0:W].rearrange("p (g r sc) -> p r g sc", g=GS, r=2)
            Iv = TC[:, W : 2 * W].rearrange("p (g r sc) -> p r g sc", g=GS, r=2)

            # products, bf16, all-SBUF (3 merged muls on DVE)
            M1 = prod.tile([128, 2 * GS * 256], BF16, tag="m1")  # t0=ac t1=bd
            M1v = M1[:].rearrange("p (t g sc) -> p t g sc", t=2, g=GS)
            nc.vector.tensor_mul(M1v, Jv, Iv)
            M3 = prod.tile([128, 2 * GS * 256], BF16, tag="m3")  # t0=ab t1=cd
            M3v = M3[:].rearrange("p (t g sc) -> p t g sc", t=2, g=GS)
            nc.vector.tensor_mul(M3v, TC5[:, :, :, 0, :], TC5[:, :, :, 1, :])
            M2 = prod.tile([128, 2 * GS * 256], BF16, tag="m2")  # t0=ad t1=bc
            M2v = M2[:].rearrange("p (t g sc) -> p t g sc", t=2, g=GS)
            nc.vector.tensor_mul(M2v, Jv, TC5[:, 1, :, ::-1, :])

            # squares from the SBUF copy: Pool bulk half, ACT the rest
            SQA = sqp.tile([128, 2 * W], BF16, tag="sqa")
            nc.gpsimd.tensor_mul(SQA[:, 0:1024], TC[:, 0:1024], TC[:, 0:1024])
            nc.scalar.activation(
                SQA[:, 1024 : 2 * W], TC[:, 1024 : 2 * W], ACTF.Square
            )

            # chunk-major reductions (PE ones-matmuls) into own PSUM pool
            SP = red.tile([128, GS * 10], F32, tag="sp")
            for g in range(GS):
                for i in range(2):
                    for r in range(2):
                        t = i * 2 + r
                        col = g * 10 + t
                        for sh in range(2):
                            off = (i * GS + g) * 512 + (2 * r + sh) * 128
                            nc.tensor.matmul(
                                SP[:, col : col + 1],
                                SQA[:, off : off + 128],
                                ones[:, 0:1],
                                start=(sh == 0),
                                stop=(sh == 1),
                            )
                for tile_, tt, t in (
                    (M3, 0, 4),
                    (M3, 1, 5),
                    (M1, 0, 6),
                    (M1, 1, 7),
                    (M2, 0, 8),
                    (M2, 1, 9),
                ):
                    col = g * 10 + t
                    for sh in range(2):
                        off = (tt * GS + g) * 256 + sh * 128
                        nc.tensor.matmul(
                            SP[:, col : col + 1],
                            tile_[:, off : off + 128],
                            ones[:, 0:1],
                            start=(sh == 0),
                            stop=(sh == 1),
                        )

            # drain the set's stats to SBUF (GPSIMD cannot touch PSUM)
            nc.vector.tensor_copy(S[:, g0 * 10 : (g0 + GS) * 10], SP[:])

        # software pipeline: transposes run 2 sets ahead of each set's body
        # so PE (in-order) never blocks the PSUM-buffer recycle
        PTq = [emit_transposes(0), emit_transposes(1)]
        for si in range(SETS):
            if si + 2 < SETS:
                PTq.append(emit_transposes(si + 2))
            emit_body(si, PTq[si])

        _epilogue(tc, nc, epi, S, out)


def _merge_act_table_loads(nc):
    """Collapse greedy per-func act-table loads into one load of the set
    that contains every function this kernel uses (copy/square/ln/exp)."""
    from concourse.hw_specs import get_activation_tables

    try:
        tables = get_activation_tables(nc.m.arch)
    except Exception:
        return
    target = None
    need = {
        ACTF.Copy,
        ACTF.Square,
        ACTF.Ln,
        ACTF.Exp,
        ACTF.Identity,
    }
    for idx, (name, funcs) in enumerate(tables.items()):
        if need.issubset(funcs):
            target = idx
            break
    if target is None:
        return
    for blk in nc.m.functions[0].blocks:
        loads = [
            inst
            for inst in blk.instructions
            if isinstance(inst, mybir.InstLoadActFuncSet)
        ]
        if not loads:
            continue
        # only safe to drop loads that carry no semaphore waits/updates
        def _sync_free(inst):
            si = inst.sync_info
            return si is None or (len(si.on_wait) == 0 and len(si.on_update) == 0)

        if not all(_sync_free(l) for l in loads[1:]):
            for l in loads:
                l.act_func_set_id = target
            continue
        loads[0].act_func_set_id = target
        keep = set(id(l) for l in loads[1:])
        insts = [i for i in blk.instructions if id(i) not in keep]
        del blk.instructions[:]
        for i in insts:
            blk.instructions.append(i)


_NC_CACHE = None


def _build_nc():
    global _NC_CACHE
    if _NC_CACHE is not None:
        return _NC_CACHE
    nc = bacc.Bacc("TRN2", target_bir_lowering=False, debug=False, num_devices=N_CORES)
    zjs = nc.dram_tensor("zjs", [ROWS, D], F32, kind="ExternalInput")
    zis = nc.dram_tensor("zis", [ROWS, D], F32, kind="ExternalInput")
    out = nc.dram_tensor("out", [128, 1], F32, kind="ExternalOutput")
    with tile.TileContext(nc) as tc:
        _trace_kernel(tc, nc, zjs.ap(), zis.ap(), out.ap())
    nc.compile()
    _NC_CACHE = nc
    return nc


def run_cores(zis, zjs, trace=False):
    """Run the SPMD kernel; returns (list of per-core out arrays, results)."""
    nc = _build_nc()
    zis = np.ascontiguousarray(np.asarray(zis, dtype=np.float32))
    zjs = np.ascontiguousarray(np.asarray(zjs, dtype=np.float32))
    in_maps = []
    for i in range(N_CORES):
        sl = slice(i * ROWS, (i + 1) * ROWS)
        in_maps.append({"zis": zis[sl], "zjs": zjs[sl]})
    res = run_bass_kernel_spmd(nc, in_maps, list(range(N_CORES)), trace=trace)
    return [r["out"] for r in res.results], res


def kernel(zis, zjs):
    outs, _ = run_cores(zis, zjs, trace=False)
    total = np.sum([o.astype(np.float64).sum() for o in outs])
    return np.asarray(total / B_FULL, dtype=np.float32)


# revision 12
# speedup vs baseline: 1.1277x; 1.1277x over previous
"""Conditional NT-Xent loss kernel V4 for Trainium2 (8 NeuronCores, SPMD).

Per chunk of 2 consecutive rows from each of zjs/zis (a,b = zjs rows; c,d = zis
rows): need 4 squared norms + 6 pairwise dots, cos_xy = s_xy/(|x||y|),
logits = 2*cos, per-row loss = lse(3 logits) - pos, total = sum / B.

V4 pipeline (vs V2 baseline): 12 staggered cast-DMAs (f32->bf16) with the
later dispatches interleaved into the set loop so Pool's in-order queue can
run square work between SWDGE-ring waits; 2-group PSUM sets with bufs=3 and
transposes emitted
two sets ahead so PE's in-order queue never blocks the PSUM recycle; one
merged copy pair (DVE head + ACT tail) materializes each set (HW allows at
most one PSUM input per compute op and GPSIMD cannot touch PSUM at all);
3 merged DVE product muls (incl. a negative-stride view for ad/bc) instead
of 12; squares split Pool/ACT as bf16 muls/Square; PE 1-col ones-matmul
reductions (~2ns each back-to-back); act-table loads merged post-compile to
a single load of natural_log_exp_and_others (saves 3x 1283ns on ACT).

Sharding: batch(chunk)-parallel across 8 cores; each core computes a partial
sum [128,1]; host sums partials and divides by B.
"""

import numpy as np

import concourse.bass as bass
import concourse.tile as tile
from concourse import bacc, masks, mybir
from concourse.bass_utils import run_bass_kernel_spmd

N_CORES = 8
B_FULL = 65536            # total rows in zis (== zjs)
ROWS = B_FULL // N_CORES  # 8192 rows per core shard
D = 256
GROUPS = 32               # groups of 128 chunks per core
GS = 2                    # groups per set: PT [128,2048] bf16 = 2 PSUM banks
SETS = GROUPS // GS
F32 = mybir.dt.float32
BF16 = mybir.dt.bfloat16
ALU = mybir.AluOpType
ACTF = mybir.ActivationFunctionType

# stats tile S[:, g*10 + t]
# t: 0=na 1=nb 2=nc 3=nd 4=ab 5=cd 6=ac 7=bd 8=ad 9=bc


def _epilogue(tc, nc, epi, S, out, ga, gb, ACCs):
    """Per-chunk softmax math on stats S[:, ga*10:gb*10]; accumulates ACC."""
    G = gb - ga
    Sv = S[:, ga * 10 : gb * 10].rearrange("p (g t) -> p g t", t=10)
    norms = Sv[:, :, 0:4]
    svals = Sv[:, :, 4:10]

    LN = epi.tile([128, G * 4], F32, tag=f"ln{ga}")
    LNv = LN[:].rearrange("p (g t) -> p g t", t=4)
    nc.scalar.activation(LNv, norms, ACTF.Ln)

    # q_xy = ln nx + ln ny (pair order ab cd ac bd ad bc)
    Q = epi.tile([128, G * 6], F32, tag=f"q{ga}")
    Qv = Q[:].rearrange("p (g t) -> p g t", t=6)
    # merged pair sums: t0,t1=(0+1, 2+3); t2,t3=(0+2, 1+3); t4,t5=(0+3, 1-2rev)
    LN4 = LN[:].rearrange("p (g t) -> p g t", t=4)
    nc.vector.tensor_add(Qv[:, :, 0:2], LN4[:, :, 0::2], LN4[:, :, 1::2])
    nc.vector.tensor_add(Qv[:, :, 2:4], LN4[:, :, 0:2], LN4[:, :, 2:4])
    nc.vector.tensor_add(Qv[:, :, 4:6], LN4[:, :, 0:2], LN4[:, :, 3:1:-1])

    # rr_xy = exp(-0.5 q) = 1/(|x||y|)
    RQ = epi.tile([128, G * 6], F32, tag=f"rq{ga}")
    RQv = RQ[:].rearrange("p (g t) -> p g t", t=6)
    nc.scalar.activation(RQv, Qv, ACTF.Exp, scale=-0.5)

    # cos_xy = s_xy * rr_xy
    C = epi.tile([128, G * 6], F32, tag=f"cos{ga}")
    Cv = C[:].rearrange("p (g t) -> p g t", t=6)
    nc.vector.tensor_mul(Cv, svals, RQv)

    # E_xy = exp(2 cos)
    E = epi.tile([128, G * 6], F32, tag=f"e{ga}")
    Ev = E[:].rearrange("p (g t) -> p g t", t=6)
    nc.scalar.activation(Ev, Cv, ACTF.Exp, scale=2.0)

    def ecol(t):
        return Ev[:, :, t : t + 1]

    # softmax denominators for the 4 rows of each chunk
    DEN = epi.tile([128, G * 4], F32, tag=f"den{ga}")
    DENv = DEN[:].rearrange("p (g t) -> p g t", t=4)
    TMP = epi.tile([128, G * 4], F32, tag=f"tmp{ga}")
    TMPv = TMP[:].rearrange("p (g t) -> p g t", t=4)
    # D0=(E0+E4)+E2, D2=(E5+E1)+E2 ; D1=(E0+E5)+E3, D3=(E4+E1)+E3
    # stage 1 merged via strided views, stage 2 adds a broadcast column
    nc.vector.tensor_add(TMPv[:, :, 0:2], Ev[:, :, 0::5], Ev[:, :, 4:0:-3])
    nc.vector.tensor_add(TMPv[:, :, 2:4], Ev[:, :, 0::4], Ev[:, :, 5:0:-4])
    nc.vector.tensor_add(
        DENv[:, :, 0:2], TMPv[:, :, 0:2], Ev[:, :, 2:3].broadcast_to([128, G, 2])
    )
    nc.vector.tensor_add(
        DENv[:, :, 2:4], TMPv[:, :, 2:4], Ev[:, :, 3:4].broadcast_to([128, G, 2])
    )

    LD = epi.tile([128, G * 4], F32, tag=f"ld{ga}")
    LDv = LD[:].rearrange("p (g t) -> p g t", t=4)
    nc.scalar.activation(LDv, DENv, ACTF.Ln)

    LG = epi.tile([128, G], F32, tag=f"lg{ga}")
    nc.vector.reduce_sum(
        LG[:].rearrange("p (g o) -> p g o", o=1), LDv, axis=mybir.AxisListType.X
    )

    T1 = epi.tile([128, G], F32, tag=f"t1{ga}")
    nc.vector.tensor_add(
        T1[:].rearrange("p (g o) -> p g o", o=1), Cv[:, :, 2:3], Cv[:, :, 3:4]
    )

    # loss per chunk-col = LG - 4*T1
    LC = epi.tile([128, G], F32, tag=f"lc{ga}")
    nc.vector.scalar_tensor_tensor(
        out=LC[:], in0=T1[:], scalar=-4.0, in1=LG[:], op0=ALU.mult, op1=ALU.add
    )

    ACC = epi.tile([128, 1], F32, tag=f"acc{ga}")
    nc.vector.reduce_sum(ACC[:], LC[:], axis=mybir.AxisListType.X)
    ACCs.append(ACC)
    if len(ACCs) == 2:
        TOT = epi.tile([128, 1], F32, tag="tot")
        nc.vector.tensor_add(TOT[:], ACCs[0][:], ACCs[1][:])
        nc.sync.dma_start(out=out, in_=TOT[:])


def _trace_kernel(tc, nc, zjs, zis, out):
    # chunk-major natural views: partition = chunk-in-group, free = (g, two, f)
    zjs_v = zjs.rearrange("(g p two) f -> p g (two f)", p=128, two=2)
    zis_v = zis.rearrange("(g p two) f -> p g (two f)", p=128, two=2)

    with (
        tc.tile_pool(name="consts", bufs=1) as consts,
        tc.tile_pool(name="nat", bufs=1) as nat,
        tc.tile_pool(name="tp", bufs=3, space="PSUM") as tp,
        tc.tile_pool(name="red", bufs=2, space="PSUM") as red,
        tc.tile_pool(name="prod", bufs=6) as prod,
        tc.tile_pool(name="sq", bufs=4) as sqp,
        tc.tile_pool(name="stats", bufs=1) as stats,
        tc.tile_pool(name="epi", bufs=1) as epi,
    ):
        ident = consts.tile([128, 128], BF16)
        masks.make_identity(nc, ident[:])
        ones = consts.tile([128, 1], BF16)
        nc.gpsimd.memset(ones[:], 1.0)

        # full-shard natural loads, bf16 cast on DMA; staggered spans so the
        # first set can start early while later, larger loads stream in
        ZJ = nat.tile([128, GROUPS * 512], BF16)
        ZI = nat.tile([128, GROUPS * 512], BF16)
        spans = [(0, 2), (2, 6), (6, 11), (11, 17), (17, 24), (24, 32)]

        def emit_load(d):
            g0, g1 = spans[d]
            for Z, zv in ((ZJ, zjs_v), (ZI, zis_v)):
                nc.gpsimd.dma_start(
                    out=Z[:, g0 * 512 : g1 * 512].rearrange(
                        "p (g tf) -> p g tf", g=g1 - g0
                    ),
                    in_=zv[:, g0:g1, :],
                )

        # first two spans upfront; later dispatches are interleaved into the
        # set loop so Pool's in-order queue doesn't block its square work on
        # SWDGE ring space between back-to-back dispatches
        emit_load(0)
        emit_load(1)
        emit_load(2)

        S = stats.tile([128, GROUPS * 10], F32)
        W = GS * 512  # per-input col width in PT

        def emit_transposes(si):
            PT = tp.tile([128, 2 * W], BF16, tag="pt")
            for i, Z in ((0, ZJ), (1, ZI)):
                for g in range(GS):
                    base = (i * GS + g) * 512
                    nbase = (si * GS + g) * 512
                    for k in range(4):
                        nc.tensor.transpose(
                            PT[:, base + 128 * k : base + 128 * (k + 1)],
                            Z[:, nbase + 128 * k : nbase + 128 * (k + 1)],
                            ident[:],
                        )
            return PT

        def emit_body(si, PT):
            g0 = si * GS

            # materialize PSUM->SBUF: hardware allows at most ONE PSUM input
            # per compute instruction, so products need an SBUF copy. Split
            # the copy across ACT (bulk) and DVE (tail); squares then read
            # the SBUF copy so they can run on DVE/Pool too.
            TC = prod.tile([128, 2 * W], BF16, tag="tc")
            nc.vector.tensor_copy(TC[:, 0:768], PT[:, 0:768])
            nc.scalar.activation(TC[:, 768 : 2 * W], PT[:, 768 : 2 * W], ACTF.Copy)

            TC5 = TC[:].rearrange("p (i g r sc) -> p i g r sc", i=2, g=GS, r=2)
            Jv = TC[:, 0:W].rearrange("p (g r sc) -> p r g sc", g=GS, r=2)
            Iv = TC[:, W : 2 * W].rearrange("p (g r sc) -> p r g sc", g=GS, r=2)

            # products, bf16, all-SBUF (3 merged muls on DVE)
            M1 = prod.tile([128, 2 * GS * 256], BF16, tag="m1")  # t0=ac t1=bd
            M1v = M1[:].rearrange("p (t g sc) -> p t g sc", t=2, g=GS)
            nc.vector.tensor_mul(M1v, Jv, Iv)
            M3 = prod.tile([128, 2 * GS * 256], BF16, tag="m3")  # t0=ab t1=cd
            M3v = M3[:].rearrange("p (t g sc) -> p t g sc", t=2, g=GS)
            nc.vector.tensor_mul(M3v, TC5[:, :, :, 0, :], TC5[:, :, :, 1, :])
            M2 = prod.tile([128, 2 * GS * 256], BF16, tag="m2")  # t0=ad t1=bc
            M2v = M2[:].rearrange("p (t g sc) -> p t g sc", t=2, g=GS)
            nc.vector.tensor_mul(M2v, Jv, TC5[:, 1, :, ::-1, :])

            # squares from the SBUF copy: Pool bulk half, ACT the rest
            SQA = sqp.tile([128, 2 * W], BF16, tag="sqa")
            nc.gpsimd.tensor_mul(SQA[:, 0:768], TC[:, 0:768], TC[:, 0:768])
            nc.scalar.activation(
                SQA[:, 768 : 2 * W], TC[:, 768 : 2 * W], ACTF.Square
            )

            # chunk-major reductions (PE ones-matmuls) into own PSUM pool
            SP = red.tile([128, GS * 10], F32, tag="sp")
            for g in range(GS):
                for i in range(2):
                    for r in range(2):
                        t = i * 2 + r
                        col = g * 10 + t
                        for sh in range(2):
                            off = (i * GS + g) * 512 + (2 * r + sh) * 128
                            nc.tensor.matmul(
                                SP[:, col : col + 1],
                                SQA[:, off : off + 128],
                                ones[:, 0:1],
                                start=(sh == 0),
                                stop=(sh == 1),
                            )
                for tile_, tt, t in (
                    (M3, 0, 4),
                    (M3, 1, 5),
                    (M1, 0, 6),
                    (M1, 1, 7),
                    (M2, 0, 8),
                    (M2, 1, 9),
                ):
                    col = g * 10 + t
                    for sh in range(2):
                        off = (tt * GS + g) * 256 + sh * 128
                        nc.tensor.matmul(
                            SP[:, col : col + 1],
                            tile_[:, off : off + 128],
                            ones[:, 0:1],
                            start=(sh == 0),
                            stop=(sh == 1),
                        )

            # drain the set's stats to SBUF (GPSIMD cannot touch PSUM)
            nc.vector.tensor_copy(S[:, g0 * 10 : (g0 + GS) * 10], SP[:])

        # software pipeline: transposes run 2 sets ahead of each set's body
        # so PE (in-order) never blocks the PSUM-buffer recycle
        dispatch_at = {1: 3, 3: 4, 6: 5}
        PTq = [emit_transposes(0), emit_transposes(1)]
        for si in range(SETS):
            if si + 2 < SETS:
                PTq.append(emit_transposes(si + 2))
            emit_body(si, PTq[si])
            if si in dispatch_at:
                emit_load(dispatch_at[si])

        # two independent half-epilogues emitted back-to-back at the end:
        # their ACT/DVE dependency chains interleave in anti-phase, halving
        # the serial cross-engine latency of the tail
        ACCs = []
        _epilogue(tc, nc, epi, S, out, 0, GROUPS // 2, ACCs)
        _epilogue(tc, nc, epi, S, out, GROUPS // 2, GROUPS, ACCs)


def _merge_act_table_loads(nc):
    """Collapse greedy per-func act-table loads into one load of the set
    that contains every function this kernel uses (copy/square/ln/exp)."""
    from concourse.hw_specs import get_activation_tables

    try:
        tables = get_activation_tables(nc.m.arch)
    except Exception:
        return
    target = None
    need = {
        ACTF.Copy,
        ACTF.Square,
        ACTF.Ln,
        ACTF.Exp,
        ACTF.Identity,
    }
    for idx, (name, funcs) in enumerate(tables.items()):
        if need.issubset(funcs):
            target = idx
            break
    if target is None:
        return
    for blk in nc.m.functions[0].blocks:
        loads = [
            inst
            for inst in blk.instructions
            if isinstance(inst, mybir.InstLoadActFuncSet)
        ]
        if not loads:
            continue
        # only safe to drop loads that carry no semaphore waits/updates
        def _sync_free(inst):
            si = inst.sync_info
            return si is None or (len(si.on_wait) == 0 and len(si.on_update) == 0)

        if not all(_sync_free(l) for l in loads[1:]):
            for l in loads:
                l.act_func_set_id = target
            continue
        loads[0].act_func_set_id = target
        keep = set(id(l) for l in loads[1:])
        insts = [i for i in blk.instructions if id(i) not in keep]
        del blk.instructions[:]
        for i in insts:
            blk.instructions.append(i)


_NC_CACHE = None


def _build_nc():
    global _NC_CACHE
    if _NC_CACHE is not None:
        return _NC_CACHE
    nc = bacc.Bacc("TRN2", target_bir_lowering=False, debug=False, num_devices=N_CORES)
    zjs = nc.dram_tensor("zjs", [ROWS, D], F32, kind="ExternalInput")
    zis = nc.dram_tensor("zis", [ROWS, D], F32, kind="ExternalInput")
    out = nc.dram_tensor("out", [128, 1], F32, kind="ExternalOutput")
    with tile.TileContext(nc) as tc:
        _trace_kernel(tc, nc, zjs.ap(), zis.ap(), out.ap())
    nc.compile()
    _merge_act_table_loads(nc)
    _NC_CACHE = nc
    return nc


def run_cores(zis, zjs, trace=False):
    """Run the SPMD kernel; returns (list of per-core out arrays, results)."""
    nc = _build_nc()
    zis = np.ascontiguousarray(np.asarray(zis, dtype=np.float32))
    zjs = np.ascontiguousarray(np.asarray(zjs, dtype=np.float32))
    in_maps = []
    for i in range(N_CORES):
        sl = slice(i * ROWS, (i + 1) * ROWS)
        in_maps.append({"zis": zis[sl], "zjs": zjs[sl]})
    res = run_bass_kernel_spmd(nc, in_maps, list(range(N_CORES)), trace=trace)
    return [r["out"] for r in res.results], res


def kernel(zis, zjs):
    outs, _ = run_cores(zis, zjs, trace=False)
    total = np.sum([o.astype(np.float64).sum() for o in outs])
    return np.asarray(total / B_FULL, dtype=np.float32)
